# revision 1
# baseline (speedup 1.0000x reference)
import os
import sys
import numpy as np
from contextlib import ExitStack

for _p in ("/opt/trn_rl_repo", "/root/.axon_site/_ro/trn_rl_repo"):
    if os.path.isdir(_p) and _p not in sys.path:
        sys.path.append(_p)

D = 256
H = 4
DH = 64
N_SRC = 100000
N_DST = 50000
N_EDGES = 300000
NDEV = 8
DST_PER_DEV = N_DST // NDEV  # 6250
NBLK = (DST_PER_DEV + 127) // 128  # 49
DST_PAD = NBLK * 128  # 6272

LAST_EXEC_NS = None


def _prep_host(h_src, h_dst, src_idx, dst_idx, Wq, bq, Wk, bk, Wv, bv):
    order = np.argsort(dst_idx, kind="stable")
    sdst = dst_idx[order]
    bounds = np.searchsorted(sdst, np.arange(0, N_DST + 1, DST_PER_DEV))

    per_dev = []
    C = 1
    for d in range(NDEV):
        lo, hi = int(bounds[d]), int(bounds[d + 1])
        local = (sdst[lo:hi] - d * DST_PER_DEV).astype(np.int64)
        blk = local // 128
        cnt = np.bincount(blk, minlength=NBLK)
        if cnt.max() > 0:
            C = max(C, int(np.ceil(cnt.max() / 128.0)))
        per_dev.append((lo, hi, local, blk, cnt))

    WKV = np.ascontiguousarray(
        np.concatenate([Wk.T, Wv.T], axis=1).astype(np.float32).reshape(2, 128, 512))
    WQ = np.ascontiguousarray(Wq.T.astype(np.float32).reshape(2, 128, 256))
    BKV = np.concatenate([bk, bv]).astype(np.float32).reshape(1, 512)
    BQ = bq.astype(np.float32).reshape(1, 256)
    has_bias = bool(np.any(BKV) or np.any(BQ))

    nchunks = NBLK * C
    E_pad = nchunks * 128
    in_maps = []
    for d in range(NDEV):
        lo, hi, local, blk, cnt = per_dev[d]
        starts = np.concatenate([[0], np.cumsum(cnt)[:-1]])
        pos = np.arange(hi - lo) - starts[blk]
        slot = blk * (C * 128) + pos

        eids = order[lo:hi]
        Xf = np.zeros((E_pad, D), np.float32)
        Xf[slot] = h_src[src_idx[eids]]
        dloc = np.full(E_pad, 128, np.int64)
        dloc[slot] = local % 128
        A2f = np.zeros((E_pad, 129), np.float32)
        A2f[np.arange(E_pad), dloc] = 1.0
        A2 = np.ascontiguousarray(A2f[:, :128].reshape(nchunks, 128, 128))
        A1 = np.ascontiguousarray(A2.transpose(0, 2, 1))
        X = np.ascontiguousarray(Xf.reshape(nchunks, 128, D).transpose(0, 2, 1))

        hd = np.zeros((DST_PAD, D), np.float32)
        hd[:DST_PER_DEV] = h_dst[d * DST_PER_DEV:(d + 1) * DST_PER_DEV]
        HD = np.ascontiguousarray(hd.reshape(NBLK, 128, 2, 128).transpose(0, 2, 3, 1))

        in_maps.append({"X": X, "A1": A1, "A2": A2, "HD": HD,
                        "WKV": WKV, "WQ": WQ, "BKV": BKV, "BQ": BQ})
    return in_maps, C, has_bias


def _build(C, has_bias):
    from concourse import bacc, bass, mybir, tile

    F32 = mybir.dt.float32
    nchunks = NBLK * C
    nc = bacc.Bacc(trn_type="TRN2")
    X_d = nc.dram_tensor("X", [nchunks, D, 128], F32, kind="ExternalInput")
    A1_d = nc.dram_tensor("A1", [nchunks, 128, 128], F32, kind="ExternalInput")
    A2_d = nc.dram_tensor("A2", [nchunks, 128, 128], F32, kind="ExternalInput")
    HD_d = nc.dram_tensor("HD", [NBLK, 2, 128, 128], F32, kind="ExternalInput")
    WKV_d = nc.dram_tensor("WKV", [2, 128, 512], F32, kind="ExternalInput")
    WQ_d = nc.dram_tensor("WQ", [2, 128, 256], F32, kind="ExternalInput")
    BKV_d = nc.dram_tensor("BKV", [1, 512], F32, kind="ExternalInput")
    BQ_d = nc.dram_tensor("BQ", [1, 256], F32, kind="ExternalInput")
    out_d = nc.dram_tensor("out", [NBLK, 128, 256], F32, kind="ExternalOutput")

    Copy = mybir.ActivationFunctionType.Copy
    Exp = mybir.ActivationFunctionType.Exp
    mult = mybir.AluOpType.mult
    addop = mybir.AluOpType.add
    maxop = mybir.AluOpType.max

    with ExitStack() as ctx:
        tc = ctx.enter_context(tile.TileContext(nc))
        cpool = ctx.enter_context(tc.tile_pool(name="const", bufs=1))
        bpool = ctx.enter_context(tc.tile_pool(name="blk", bufs=2))
        kpool = ctx.enter_context(tc.tile_pool(name="chunk", bufs=3))
        qpp = ctx.enter_context(tc.tile_pool(name="qps", bufs=1, space="PSUM"))
        upp = ctx.enter_context(tc.tile_pool(name="ups", bufs=2, space="PSUM"))
        kpp = ctx.enter_context(tc.tile_pool(name="kvp", bufs=2, space="PSUM"))
        gpp = ctx.enter_context(tc.tile_pool(name="qgp", bufs=2, space="PSUM"))

        wkv_sb = cpool.tile([128, 2, 512], F32)
        nc.sync.dma_start(out=wkv_sb, in_=WKV_d.rearrange("s p e -> p s e"))
        wq_sb = cpool.tile([128, 2, 256], F32)
        nc.sync.dma_start(out=wq_sb, in_=WQ_d.rearrange("s p e -> p s e"))
        if has_bias:
            ones_sb = cpool.tile([1, 128], F32)
            nc.vector.memset(ones_sb, 1.0)
            bkv_sb = cpool.tile([1, 512], F32)
            nc.sync.dma_start(out=bkv_sb, in_=BKV_d)
            bq_sb = cpool.tile([1, 256], F32)
            nc.sync.dma_start(out=bq_sb, in_=BQ_d)

        for b in range(NBLK):
            hd_sb = bpool.tile([128, 2, 128], F32)
            nc.sync.dma_start(out=hd_sb, in_=HD_d[b].rearrange("s c d -> c s d"))
            xblk = bpool.tile([128, C, 2, 128], F32)
            nc.sync.dma_start(
                out=xblk,
                in_=X_d[b * C:(b + 1) * C].rearrange("c (s p) e -> p c s e", s=2))
            a1 = bpool.tile([128, C, 128], F32)
            nc.sync.dma_start(out=a1, in_=A1_d[b * C:(b + 1) * C].rearrange("c p e -> p c e"))
            a2 = bpool.tile([128, C, 128], F32)
            nc.sync.dma_start(out=a2, in_=A2_d[b * C:(b + 1) * C].rearrange("c p e -> p c e"))

            qps = qpp.tile([128, 256], F32)
            nc.tensor.matmul(qps, hd_sb[:, 0, :], wq_sb[:, 0, :],
                             start=True, stop=False)
            nc.tensor.matmul(qps, hd_sb[:, 1, :], wq_sb[:, 1, :],
                             start=False, stop=not has_bias)
            if has_bias:
                nc.tensor.matmul(qps, ones_sb, bq_sb, start=False, stop=True)
            q_sb = bpool.tile([128, 256], F32)
            nc.scalar.activation(q_sb, qps, Copy)

            ups = upp.tile([128, 260], F32)
            for c in range(C):
                kv = kpp.tile([128, 512], F32)
                nc.tensor.matmul(kv, xblk[:, c, 0, :], wkv_sb[:, 0, :],
                                 start=True, stop=False)
                nc.tensor.matmul(kv, xblk[:, c, 1, :], wkv_sb[:, 1, :],
                                 start=False, stop=not has_bias)
                if has_bias:
                    nc.tensor.matmul(kv, ones_sb, bkv_sb, start=False, stop=True)
                qg = gpp.tile([128, 256], F32)
                nc.tensor.matmul(qg, a1[:, c, :], q_sb, start=True, stop=True)
                qg_sb = kpool.tile([128, 256], F32)
                nc.scalar.activation(qg_sb, qg, Copy)
                prod = kpool.tile([128, 256], F32)
                nc.vector.tensor_tensor(prod, kv[:, 0:256], qg_sb, mult)
                sc = kpool.tile([128, 4], F32)
                nc.vector.tensor_reduce(sc, prod.rearrange("p (h d) -> p h d", h=4),
                                        mybir.AxisListType.X, addop)
                es = kpool.tile([128, 4], F32)
                nc.scalar.activation(es, sc, Exp, scale=0.125)
                pcat = kpool.tile([128, 260], F32)
                nc.vector.tensor_scalar(pcat[:, 256:260], es, 0.0, None, addop)
                for h in range(H):
                    nc.vector.tensor_scalar(
                        pcat[:, h * 64:(h + 1) * 64],
                        kv[:, 256 + h * 64:256 + (h + 1) * 64],
                        es[:, h:h + 1], None, mult)
                nc.tensor.matmul(ups, a2[:, c, :], pcat,
                                 start=(c == 0), stop=(c == C - 1))

            s_sb = bpool.tile([128, 4], F32)
            nc.vector.tensor_scalar(s_sb, ups[:, 256:260], 1e-30, None, maxop)
            r_sb = bpool.tile([128, 4], F32)
            nc.vector.reciprocal(r_sb, s_sb)
            o_sb = bpool.tile([128, 256], F32)
            for h in range(H):
                nc.vector.tensor_scalar(o_sb[:, h * 64:(h + 1) * 64],
                                        ups[:, h * 64:(h + 1) * 64],
                                        r_sb[:, h:h + 1], None, mult)
            nc.sync.dma_start(out=out_d[b], in_=o_sb)
    return nc


def _emulate(in_maps, C):
    outs = []
    for m in in_maps:
        X, A1, A2, HD = m["X"], m["A1"], m["A2"], m["HD"]
        WKV, WQ, BKV, BQ = m["WKV"], m["WQ"], m["BKV"], m["BQ"]
        out = np.zeros((NBLK, 128, 256), np.float32)
        for b in range(NBLK):
            hd = HD[b]
            Q = hd[0].T @ WQ[0] + hd[1].T @ WQ[1] + BQ
            U = np.zeros((128, 260), np.float32)
            for c in range(C):
                i = b * C + c
                x = X[i]
                kv = x[:128].T @ WKV[0] + x[128:].T @ WKV[1] + BKV
                qg = A1[i].T @ Q
                sc = (kv[:, :256] * qg).reshape(128, 4, 64).sum(-1)
                p = np.exp(sc * 0.125).astype(np.float32)
                pv = (kv[:, 256:].reshape(128, 4, 64) * p[:, :, None]).reshape(128, 256)
                U += A2[i].T @ np.concatenate([pv, p], axis=1)
            r = 1.0 / np.maximum(U[:, 256:260], 1e-30)
            out[b] = (U[:, :256].reshape(128, 4, 64) * r[:, :, None]).reshape(128, 256)
        outs.append({"out": out})
    return outs


def kernel(**inputs):
    global LAST_EXEC_NS
    h_src = np.asarray(inputs["h_src"], np.float32)
    h_dst = np.asarray(inputs["h_dst"], np.float32)
    src_idx = np.asarray(inputs["src_idx"]).astype(np.int64)
    dst_idx = np.asarray(inputs["dst_idx"]).astype(np.int64)
    Wq = np.asarray(inputs["Wq"], np.float32)
    bq = np.asarray(inputs["bq"], np.float32)
    Wk = np.asarray(inputs["Wk"], np.float32)
    bk = np.asarray(inputs["bk"], np.float32)
    Wv = np.asarray(inputs["Wv"], np.float32)
    bv = np.asarray(inputs["bv"], np.float32)

    in_maps, C, has_bias = _prep_host(h_src, h_dst, src_idx, dst_idx,
                                      Wq, bq, Wk, bk, Wv, bv)

    if os.environ.get("KERNEL_EMULATE"):
        results = _emulate(in_maps, C)
    else:
        from concourse.bass_utils import run_bass_kernel_spmd
        nc = _build(C, has_bias)
        nc.finalize()
        res = run_bass_kernel_spmd(
            nc, in_maps, core_ids=list(range(NDEV)),
            trace=bool(os.environ.get("KERNEL_TRACE")))
        results = res.results
        LAST_EXEC_NS = res.exec_time_ns

    parts = [np.asarray(r["out"]).reshape(DST_PAD, 256)[:DST_PER_DEV]
             for r in results]
    return np.ascontiguousarray(np.concatenate(parts, axis=0).astype(np.float32))



# revision 4
# speedup vs baseline: 11.3711x; 11.3711x over previous
import os
import sys
import numpy as np
from contextlib import ExitStack

for _p in ("/opt/trn_rl_repo", "/root/.axon_site/_ro/trn_rl_repo"):
    if os.path.isdir(_p) and _p not in sys.path:
        sys.path.append(_p)

import ml_dtypes

BF16 = ml_dtypes.bfloat16

D = 256
H = 4
DH = 64
N_SRC = 100000
N_DST = 50000
N_EDGES = 300000
NDEV = 8
DST_PER_DEV = N_DST // NDEV  # 6250
NBLK = (DST_PER_DEV + 127) // 128  # 49
DST_PAD = NBLK * 128  # 6272

LAST_EXEC_NS = None


def _prep_host(h_src, h_dst, src_idx, dst_idx, Wq, bq, Wk, bk, Wv, bv):
    order = np.argsort(dst_idx, kind="stable")
    sdst = dst_idx[order]
    bounds = np.searchsorted(sdst, np.arange(0, N_DST + 1, DST_PER_DEV))

    per_dev = []
    C = 1
    for d in range(NDEV):
        lo, hi = int(bounds[d]), int(bounds[d + 1])
        local = (sdst[lo:hi] - d * DST_PER_DEV).astype(np.int64)
        blk = local // 128
        cnt = np.bincount(blk, minlength=NBLK)
        if cnt.max() > 0:
            C = max(C, int(np.ceil(cnt.max() / 128.0)))
        per_dev.append((lo, hi, local, blk, cnt))

    WKVT = np.ascontiguousarray(
        np.concatenate([Wk.T, Wv.T], axis=1).reshape(2, 128, 512).transpose(1, 0, 2)
    ).astype(BF16)
    WQT = np.ascontiguousarray(
        Wq.T.reshape(2, 128, 256).transpose(1, 0, 2)).astype(BF16)
    BKV = np.concatenate([bk, bv]).astype(BF16).reshape(1, 512)
    BQ = bq.astype(BF16).reshape(1, 256)
    has_bias = bool(np.any(bk) or np.any(bv) or np.any(bq))
    IOTA = np.tile(np.arange(128, dtype=np.float32), (128, 1))
    IDENT = np.eye(128, dtype=np.float32)

    h_src_bf = h_src.astype(BF16)
    h_dst_bf = h_dst.astype(BF16)

    nchunks = NBLK * C
    E_pad = nchunks * 128
    in_maps = []
    for d in range(NDEV):
        lo, hi, local, blk, cnt = per_dev[d]
        starts = np.concatenate([[0], np.cumsum(cnt)[:-1]])
        pos = np.arange(hi - lo) - starts[blk]
        slot = blk * (C * 128) + pos

        eids = order[lo:hi]
        Xf = np.zeros((E_pad, D), BF16)
        Xf[slot] = h_src_bf[src_idx[eids]]
        # [128(emb-half), nchunks, 2, 128(slot)]: partition = embedding so the
        # KV matmul contracts over it; each partition DMAs long contiguous rows
        XT = np.ascontiguousarray(
            Xf.reshape(nchunks, 128, 2, 128).transpose(3, 0, 2, 1))

        dl = np.full((NBLK, C, 128), 128.0, np.float32)
        dl.reshape(E_pad)[slot] = (local % 128).astype(np.float32)
        DLOC = np.ascontiguousarray(dl.transpose(2, 0, 1))  # [128, NBLK, C]

        hd = np.zeros((DST_PAD, D), BF16)
        hd[:DST_PER_DEV] = h_dst_bf[d * DST_PER_DEV:(d + 1) * DST_PER_DEV]
        # [128(emb), NBLK, 2, 128(dst)]
        HDT = np.ascontiguousarray(
            hd.reshape(NBLK, 128, 2, 128).transpose(3, 0, 2, 1))

        in_maps.append({"X": XT, "DLOC": DLOC, "HD": HDT,
                        "WKV": WKVT, "WQ": WQT, "BKV": BKV, "BQ": BQ,
                        "IOTA": IOTA, "IDENT": IDENT})
    return in_maps, C, has_bias


def _build(C, has_bias):
    from concourse import bacc, bass, mybir, tile

    F32 = mybir.dt.float32
    BF = mybir.dt.bfloat16
    nchunks = NBLK * C
    nc = bacc.Bacc(trn_type="TRN2")
    X_d = nc.dram_tensor("X", [128, nchunks, 2, 128], BF, kind="ExternalInput")
    DL_d = nc.dram_tensor("DLOC", [128, NBLK, C], F32, kind="ExternalInput")
    HD_d = nc.dram_tensor("HD", [128, NBLK, 2, 128], BF, kind="ExternalInput")
    WKV_d = nc.dram_tensor("WKV", [128, 2, 512], BF, kind="ExternalInput")
    WQ_d = nc.dram_tensor("WQ", [128, 2, 256], BF, kind="ExternalInput")
    BKV_d = nc.dram_tensor("BKV", [1, 512], BF, kind="ExternalInput")
    BQ_d = nc.dram_tensor("BQ", [1, 256], BF, kind="ExternalInput")
    IOTA_d = nc.dram_tensor("IOTA", [128, 128], F32, kind="ExternalInput")
    IDENT_d = nc.dram_tensor("IDENT", [128, 128], F32, kind="ExternalInput")
    out_d = nc.dram_tensor("out", [NBLK, 128, 256], BF, kind="ExternalOutput")

    Copy = mybir.ActivationFunctionType.Copy
    Exp = mybir.ActivationFunctionType.Exp
    mult = mybir.AluOpType.mult
    addop = mybir.AluOpType.add
    maxop = mybir.AluOpType.max
    iseq = mybir.AluOpType.is_equal

    with ExitStack() as ctx:
        tc = ctx.enter_context(tile.TileContext(nc))
        cpool = ctx.enter_context(tc.tile_pool(name="const", bufs=1))
        bpool = ctx.enter_context(tc.tile_pool(name="blk", bufs=2))
        kpool = ctx.enter_context(tc.tile_pool(name="chunk", bufs=3))
        qpp = ctx.enter_context(tc.tile_pool(name="qps", bufs=1, space="PSUM"))
        upp = ctx.enter_context(tc.tile_pool(name="ups", bufs=2, space="PSUM"))
        kpp = ctx.enter_context(tc.tile_pool(name="kvp", bufs=2, space="PSUM"))
        gpp = ctx.enter_context(tc.tile_pool(name="qgp", bufs=2, space="PSUM"))
        app = ctx.enter_context(tc.tile_pool(name="a1p", bufs=1, space="PSUM"))

        wkv_sb = cpool.tile([128, 2, 512], BF)
        nc.sync.dma_start(out=wkv_sb, in_=WKV_d)
        wq_sb = cpool.tile([128, 2, 256], BF)
        nc.sync.dma_start(out=wq_sb, in_=WQ_d)
        iota_sb = cpool.tile([128, 128], F32)
        nc.sync.dma_start(out=iota_sb, in_=IOTA_d)
        ident_sb = cpool.tile([128, 128], F32)
        nc.sync.dma_start(out=ident_sb, in_=IDENT_d)
        dloc_sb = cpool.tile([128, NBLK, C], F32)
        nc.sync.dma_start(out=dloc_sb, in_=DL_d)
        if has_bias:
            ones_sb = cpool.tile([1, 128], BF)
            nc.vector.memset(ones_sb, 1.0)
            bkv_sb = cpool.tile([1, 512], BF)
            nc.sync.dma_start(out=bkv_sb, in_=BKV_d)
            bq_sb = cpool.tile([1, 256], BF)
            nc.sync.dma_start(out=bq_sb, in_=BQ_d)

        for b in range(NBLK):
            hd_sb = bpool.tile([128, 2, 128], BF)
            nc.sync.dma_start(out=hd_sb, in_=HD_d[:, b])
            xblk = bpool.tile([128, C, 2, 128], BF)
            nc.sync.dma_start(out=xblk, in_=X_d[:, b * C:(b + 1) * C])

            qps = qpp.tile([128, 256], F32)
            nc.tensor.matmul(qps, hd_sb[:, 0, :], wq_sb[:, 0, :],
                             start=True, stop=False)
            nc.tensor.matmul(qps, hd_sb[:, 1, :], wq_sb[:, 1, :],
                             start=False, stop=not has_bias)
            if has_bias:
                nc.tensor.matmul(qps, ones_sb, bq_sb, start=False, stop=True)
            q_sb = bpool.tile([128, 256], F32)
            nc.scalar.activation(q_sb, qps, Copy)

            ups = upp.tile([128, 260], F32)
            for c in range(C):
                kv = kpp.tile([128, 512], F32)
                nc.tensor.matmul(kv, xblk[:, c, 0, :], wkv_sb[:, 0, :],
                                 start=True, stop=False)
                nc.tensor.matmul(kv, xblk[:, c, 1, :], wkv_sb[:, 1, :],
                                 start=False, stop=not has_bias)
                if has_bias:
                    nc.tensor.matmul(kv, ones_sb, bkv_sb, start=False, stop=True)
                a2_sb = kpool.tile([128, 128], F32)
                nc.vector.tensor_scalar(a2_sb, iota_sb, dloc_sb[:, b, c:c + 1],
                                        None, iseq)
                a1ps = app.tile([128, 128], F32)
                nc.tensor.matmul(a1ps, a2_sb, ident_sb, start=True, stop=True)
                a1_sb = kpool.tile([128, 128], F32)
                nc.scalar.activation(a1_sb, a1ps, Copy)
                qg = gpp.tile([128, 256], F32)
                nc.tensor.matmul(qg, a1_sb, q_sb, start=True, stop=True)
                qg_sb = kpool.tile([128, 256], F32)
                nc.scalar.activation(qg_sb, qg, Copy)
                prod = kpool.tile([128, 256], F32)
                nc.vector.tensor_tensor(prod, kv[:, 0:256], qg_sb, mult)
                sc = kpool.tile([128, 4], F32)
                nc.vector.tensor_reduce(sc, prod.rearrange("p (h d) -> p h d", h=4),
                                        mybir.AxisListType.X, addop)
                es = kpool.tile([128, 4], F32)
                nc.scalar.activation(es, sc, Exp, scale=0.125)
                pcat = kpool.tile([128, 260], F32)
                nc.vector.tensor_scalar(pcat[:, 256:260], es, 0.0, None, addop)
                for h in range(H):
                    nc.vector.tensor_scalar(
                        pcat[:, h * 64:(h + 1) * 64],
                        kv[:, 256 + h * 64:256 + (h + 1) * 64],
                        es[:, h:h + 1], None, mult)
                nc.tensor.matmul(ups, a2_sb, pcat,
                                 start=(c == 0), stop=(c == C - 1))

            s_sb = bpool.tile([128, 4], F32)
            nc.vector.tensor_scalar(s_sb, ups[:, 256:260], 1e-30, None, maxop)
            r_sb = bpool.tile([128, 4], F32)
            nc.vector.reciprocal(r_sb, s_sb)
            o_sb = bpool.tile([128, 256], BF)
            for h in range(H):
                nc.vector.tensor_scalar(o_sb[:, h * 64:(h + 1) * 64],
                                        ups[:, h * 64:(h + 1) * 64],
                                        r_sb[:, h:h + 1], None, mult)
            nc.sync.dma_start(out=out_d[b], in_=o_sb)
    return nc


def _emulate(in_maps, C, has_bias):
    nchunks = NBLK * C
    outs = []
    for m in in_maps:
        X = m["X"].astype(np.float32)      # [128, nchunks, 2, 128]
        DL = m["DLOC"]                     # [128, NBLK, C]
        HDt = m["HD"].astype(np.float32)   # [128, NBLK, 2, 128]
        WKV = m["WKV"].astype(np.float32)  # [128, 2, 512]
        WQ = m["WQ"].astype(np.float32)
        BKV = m["BKV"].astype(np.float32)
        BQ = m["BQ"].astype(np.float32)
        iota = np.arange(128, dtype=np.float32)
        out = np.zeros((NBLK, 128, 256), np.float32)
        for b in range(NBLK):
            hd = HDt[:, b]  # [128e, 2, 128d]
            Q = hd[:, 0, :].T @ WQ[:, 0, :] + hd[:, 1, :].T @ WQ[:, 1, :]
            if has_bias:
                Q = Q + BQ
            U = np.zeros((128, 260), np.float32)
            for c in range(C):
                x = X[:, b * C + c]  # [128e, 2, 128slot]
                kv = np.einsum('es,ef->sf', x[:, 0, :], WKV[:, 0, :]) + \
                     np.einsum('es,ef->sf', x[:, 1, :], WKV[:, 1, :])
                if has_bias:
                    kv = kv + BKV
                dloc = DL[:, b, c]  # [128slot]
                a2 = (iota[None, :] == dloc[:, None]).astype(np.float32)
                qg = a2 @ Q  # gather Q rows to slots ([slot,dst]@[dst,256])
                sc = (kv[:, :256] * qg).reshape(128, 4, 64).sum(-1)
                p = np.exp(sc * 0.125).astype(np.float32)
                pv = (kv[:, 256:].reshape(128, 4, 64) * p[:, :, None]).reshape(128, 256)
                U += a2.T @ np.concatenate([pv, p], axis=1)
            r = 1.0 / np.maximum(U[:, 256:260], 1e-30)
            out[b] = (U[:, :256].reshape(128, 4, 64) * r[:, :, None]).reshape(128, 256)
        outs.append({"out": out.astype(BF16)})
    return outs


def kernel(**inputs):
    global LAST_EXEC_NS
    h_src = np.asarray(inputs["h_src"], np.float32)
    h_dst = np.asarray(inputs["h_dst"], np.float32)
    src_idx = np.asarray(inputs["src_idx"]).astype(np.int64)
    dst_idx = np.asarray(inputs["dst_idx"]).astype(np.int64)
    Wq = np.asarray(inputs["Wq"], np.float32)
    bq = np.asarray(inputs["bq"], np.float32)
    Wk = np.asarray(inputs["Wk"], np.float32)
    bk = np.asarray(inputs["bk"], np.float32)
    Wv = np.asarray(inputs["Wv"], np.float32)
    bv = np.asarray(inputs["bv"], np.float32)

    in_maps, C, has_bias = _prep_host(h_src, h_dst, src_idx, dst_idx,
                                      Wq, bq, Wk, bk, Wv, bv)

    if os.environ.get("KERNEL_EMULATE"):
        results = _emulate(in_maps, C, has_bias)
    else:
        from concourse.bass_utils import run_bass_kernel_spmd
        nc = _build(C, has_bias)
        nc.finalize()
        res = run_bass_kernel_spmd(
            nc, in_maps, core_ids=list(range(NDEV)),
            trace=bool(os.environ.get("KERNEL_TRACE")))
        results = res.results
        LAST_EXEC_NS = res.exec_time_ns

    parts = [np.asarray(r["out"]).reshape(DST_PAD, 256)[:DST_PER_DEV]
             for r in results]
    return np.ascontiguousarray(
        np.concatenate(parts, axis=0).astype(np.float32))
